# revision 1
# baseline (speedup 1.0000x reference)
"""ContinuousDeepFM Trainium2 kernel (8-core data-parallel over batch).

Math (algebraically collapsed from the reference — the [B,D,D] interaction
tensor is never materialized):
    fo  = x @ W1 + bias
    xw  = x @ W2
    so[b,j] = 0.5 * xw[b,j]^2 * t[b],  t[b] = sum_i x[b,i]^2 - (sum_i x[b,i])^2
    h   = MLP(x @ Wf)   (3 ReLU layers + final linear, weights mlp_w[i].T)
    out = fo + so + h

Sharding: batch 512 -> 64 rows per core; weights replicated. On-chip layout
is feature-major (activations stored transposed as 4 chunks of 128
partitions) so no on-chip transposes are needed; per-feature biases become
per-partition scalars. t depends only on x, so it is computed host-side in
fp64 and shipped pre-broadcast.

Precision: the output is dominated by the second-order term (RMS ~2e5 vs
~23 for fo and ~1 for h), so W2/x/so stay fp32 while the fo/deep weights
and activations run in fp8e4m3 (measured end-to-end rel err ~4e-6) at 1/4
the weight-DMA bytes.

All device inputs are host-pre-swizzled into dense [128, free] layouts so
every DMA is a contiguous 2D copy.
"""

import numpy as np
import ml_dtypes

B = 512
D = 512
NCORES = 8
BL = B // NCORES  # 64 batch rows per core
P = 128
KC = D // P  # 4 partition chunks of the feature dim

F8 = ml_dtypes.float8_e4m3
BF16 = ml_dtypes.bfloat16

_NC_CACHE = {}


def _split_multi_waits(nc, mybir):
    """This container's walrus build supports only ONE sync wait per
    instruction, but Tile's scheduler attaches several (e.g. the exit
    drain). Split extras into preceding single-wait NoOps on the same
    engine — in-order execution preserves the barrier semantics."""
    ctr = 0
    for fn in nc.m.functions:
        for blk in fn.blocks:
            insts = blk.instructions
            if not any(
                i.sync_info is not None
                and i.sync_info.on_wait
                and len(i.sync_info.on_wait) > 1
                for i in insts
            ):
                continue
            out = []
            for inst in insts:
                si = inst.sync_info
                if si is not None and si.on_wait and len(si.on_wait) > 1:
                    waits = list(si.on_wait)
                    for w in waits[:-1]:
                        ctr += 1
                        nop = mybir.InstNoOp(
                            name=f"wsplit-{ctr}-{inst.name}", ins=[], outs=[]
                        )
                        nop.engine = inst.engine
                        nop.sync_info = mybir.SyncInfo(on_wait=[w], on_update=[])
                        out.append(nop)
                    si.on_wait = [waits[-1]]
                out.append(inst)
            blk.instructions = out
    return ctr


def _build_nc():
    import concourse.bass as bass
    import concourse.mybir as mybir
    import concourse.tile as tile

    dt = mybir.dt
    f32 = dt.float32
    f8 = dt.float8e4
    Alu = mybir.AluOpType

    nc = bass.Bass("TRN2", target_bir_lowering=False, debug=False)

    x_d = nc.dram_tensor("x_d", [P, KC * BL], f32, kind="ExternalInput")
    th_d = nc.dram_tensor("th_d", [P, BL], f32, kind="ExternalInput")
    bias_d = nc.dram_tensor("bias_d", [P, 16], f32, kind="ExternalInput")
    wf_d = nc.dram_tensor("wf_d", [P, KC * D], f8, kind="ExternalInput")
    mw_d = nc.dram_tensor("mw_d", [P, 4 * KC * D], f8, kind="ExternalInput")
    w1_d = nc.dram_tensor("w1_d", [P, KC * D], f8, kind="ExternalInput")
    w2_d = nc.dram_tensor("w2_d", [P, KC * D], f32, kind="ExternalInput")
    out_d = nc.dram_tensor("out_d", [P, KC * BL], f32, kind="ExternalOutput")

    with tile.TileContext(nc) as tc:
        with (
            tc.tile_pool(name="w", bufs=1) as wpool,
            tc.tile_pool(name="act", bufs=1) as apool,
            tc.tile_pool(name="ps", bufs=1, space="PSUM") as pspool,
        ):
            # ---- input DMAs, one dense 2D copy each, split across the two
            # HWDGE rings (sync + scalar) so per-DMA completion-receipt gaps
            # overlap; deep-chain weights on ring A, w2/w1 on ring B.
            xt = apool.tile([P, KC * BL], f32, tag="xt")
            nc.sync.dma_start(xt[:], x_d.ap())
            bias_sb = apool.tile([P, 16], f32, tag="bias")
            nc.scalar.dma_start(bias_sb[:], bias_d.ap())
            th = apool.tile([P, BL], f32, tag="th")
            nc.scalar.dma_start(th[:], th_d.ap())
            wf_sb = wpool.tile([P, KC * D], f8, tag="wf")
            nc.sync.dma_start(wf_sb[:], wf_d.ap())
            w2_sb = wpool.tile([P, KC * D], f32, tag="w2")
            nc.scalar.dma_start(w2_sb[:], w2_d.ap())
            mw_sb = wpool.tile([P, 4 * KC * D], f8, tag="mw")
            for i in range(4):
                nc.sync.dma_start(
                    mw_sb[:, i * KC * D : (i + 1) * KC * D],
                    mw_d.ap()[:, i * KC * D : (i + 1) * KC * D],
                )
            w1_sb = wpool.tile([P, KC * D], f8, tag="w1")
            nc.scalar.dma_start(w1_sb[:], w1_d.ap())

            def wsl(t, kc, jc, base=0):
                return t[:, base + kc * D + jc * P : base + kc * D + (jc + 1) * P]

            def xsl(t, kc):
                return t[:, kc * BL : (kc + 1) * BL]

            # fp8 copy of x for the fo/deep matmuls
            x8 = apool.tile([P, KC * BL], f8, tag="x8")
            nc.vector.tensor_copy(x8[:], xt[:])

            # ---- deep chain (fp8): h0 = x @ Wf
            h_ps = [
                pspool.tile([P, BL], f32, tag="mm", bufs=8, name=f"h0p{j}")
                for j in range(KC)
            ]
            for kc in range(KC):
                for jc in range(KC):
                    nc.tensor.matmul(
                        h_ps[jc][:],
                        wsl(wf_sb, kc, jc),
                        xsl(x8, kc),
                        start=(kc == 0),
                        stop=(kc == KC - 1),
                    )
            h = apool.tile([P, KC * BL], f8, tag="h0")
            for jc in range(KC):
                nc.vector.tensor_copy(xsl(h, jc), h_ps[jc][:])

            # hidden layers 0..1
            for i in range(2):
                l_ps = [
                    pspool.tile([P, BL], f32, tag="mm", bufs=8, name=f"l{i}p{j}")
                    for j in range(KC)
                ]
                for kc in range(KC):
                    for jc in range(KC):
                        nc.tensor.matmul(
                            l_ps[jc][:],
                            wsl(mw_sb, kc, jc, base=i * KC * D),
                            xsl(h, kc),
                            start=(kc == 0),
                            stop=(kc == KC - 1),
                        )
                hn = apool.tile([P, KC * BL], f8, tag=f"h{i + 1}")
                for jc in range(KC):
                    nc.vector.tensor_scalar(
                        xsl(hn, jc),
                        l_ps[jc][:],
                        bias_sb[:, 4 + i * KC + jc : 5 + i * KC + jc],
                        0.0,
                        op0=Alu.add,
                        op1=Alu.max,
                    )
                h = hn

            # ---- xw = x @ W2 (fp32) ; xwsq = xw^2 on ScalarE
            xw_ps = [
                pspool.tile([P, BL], f32, tag="mm", bufs=8, name=f"xw{j}")
                for j in range(KC)
            ]
            for kc in range(KC):
                for jc in range(KC):
                    nc.tensor.matmul(
                        xw_ps[jc][:],
                        wsl(w2_sb, kc, jc),
                        xsl(xt, kc),
                        start=(kc == 0),
                        stop=(kc == KC - 1),
                    )
            xwsq = apool.tile([P, KC * BL], f32, tag="xwsq")
            for jc in range(KC):
                nc.scalar.square(xsl(xwsq, jc), xw_ps[jc][:])

            # so2 = xw^2 * (0.5*t) + btot  (btot = bias + mlp_b[3])
            so = apool.tile([P, KC * BL], f32, tag="so")
            for jc in range(KC):
                nc.vector.tensor_mul(xsl(so, jc), xsl(xwsq, jc), th[:])
            so2 = apool.tile([P, KC * BL], f32, tag="so2")
            for jc in range(KC):
                nc.vector.tensor_scalar(
                    xsl(so2, jc),
                    xsl(so, jc),
                    bias_sb[:, jc : jc + 1],
                    None,
                    op0=Alu.add,
                )

            # hidden layer 2
            i = 2
            l_ps = [
                pspool.tile([P, BL], f32, tag="mm", bufs=8, name=f"l2p{j}")
                for j in range(KC)
            ]
            for kc in range(KC):
                for jc in range(KC):
                    nc.tensor.matmul(
                        l_ps[jc][:],
                        wsl(mw_sb, kc, jc, base=i * KC * D),
                        xsl(h, kc),
                        start=(kc == 0),
                        stop=(kc == KC - 1),
                    )
            hn = apool.tile([P, KC * BL], f8, tag="h3")
            for jc in range(KC):
                nc.vector.tensor_scalar(
                    xsl(hn, jc),
                    l_ps[jc][:],
                    bias_sb[:, 4 + i * KC + jc : 5 + i * KC + jc],
                    0.0,
                    op0=Alu.add,
                    op1=Alu.max,
                )
            h = hn

            # ---- final: o = h3 @ mw[3].T + x @ W1 in one psum group
            o_ps = [
                pspool.tile([P, BL], f32, tag="mm", bufs=8, name=f"op{j}")
                for j in range(KC)
            ]
            for kc in range(KC):
                for jc in range(KC):
                    nc.tensor.matmul(
                        o_ps[jc][:],
                        wsl(mw_sb, kc, jc, base=3 * KC * D),
                        xsl(h, kc),
                        start=(kc == 0),
                        stop=False,
                    )
            for kc in range(KC):
                for jc in range(KC):
                    nc.tensor.matmul(
                        o_ps[jc][:],
                        wsl(w1_sb, kc, jc),
                        xsl(x8, kc),
                        start=False,
                        stop=(kc == KC - 1),
                    )
            out_sb = apool.tile([P, KC * BL], f32, tag="out")
            for jc in range(KC):
                nc.vector.tensor_add(xsl(out_sb, jc), o_ps[jc][:], xsl(so2, jc))

            nc.scalar.dma_start(out_d.ap(), out_sb[:])

    _split_multi_waits(nc, mybir)
    return nc


def _get_nc():
    if "nc" not in _NC_CACHE:
        _NC_CACHE["nc"] = _build_nc()
    return _NC_CACHE["nc"]


def _chunk_major(w):
    """[D, D] lhsT-layout weight -> dense [128, KC*D] chunk-major array."""
    return np.ascontiguousarray(
        w.reshape(KC, P, D).transpose(1, 0, 2).reshape(P, KC * D)
    )


def prepare_in_maps(inputs):
    x = np.asarray(inputs["x"], np.float32)
    w1 = np.asarray(inputs["first_order_weights"], np.float32)
    bias = np.asarray(inputs["bias"], np.float32)
    w2 = np.asarray(inputs["second_order_weights"], np.float32)
    wf = np.asarray(inputs["feature_weights"], np.float32)
    mw = np.asarray(inputs["mlp_w"], np.float32)
    mb = np.asarray(inputs["mlp_b"], np.float32)

    # t[b] = sum x^2 - (sum x)^2 (host, fp64), shipped as 0.5*t broadcast
    xd = x.astype(np.float64)
    t = (xd * xd).sum(1) - xd.sum(1) ** 2
    th_full = (0.5 * t).astype(np.float32)

    w2_dev = _chunk_major(w2)
    wf_dev = _chunk_major(wf).astype(F8)
    w1_dev = _chunk_major(w1).astype(F8)
    # mw[i].T is the lhsT; layer-major, then chunk-major within each layer
    mwT = mw.transpose(0, 2, 1)  # [4, D(k), D(m)]
    mw_dev = np.ascontiguousarray(
        mwT.reshape(4, KC, P, D).transpose(2, 0, 1, 3).reshape(P, 4 * KC * D)
    ).astype(F8)
    # bias_sb layout: [btot(4) | mb0(4) | mb1(4) | mb2(4)]
    btot = (bias + mb[3]).astype(np.float32).reshape(KC, P).T  # [128, 4]
    mb3 = mb[:3].astype(np.float32).reshape(3, KC, P).transpose(2, 0, 1).reshape(P, 12)
    bias_dev = np.ascontiguousarray(np.concatenate([btot, mb3], axis=1))

    in_maps = []
    for c in range(NCORES):
        xs = x[c * BL : (c + 1) * BL, :].T  # [512, 64]
        x_dev = np.ascontiguousarray(
            xs.reshape(KC, P, BL).transpose(1, 0, 2).reshape(P, KC * BL)
        )
        th_dev = np.ascontiguousarray(
            np.broadcast_to(th_full[c * BL : (c + 1) * BL], (P, BL))
        )
        in_maps.append(
            {
                "x_d": x_dev,
                "th_d": th_dev,
                "bias_d": bias_dev,
                "wf_d": wf_dev,
                "mw_d": mw_dev,
                "w1_d": w1_dev,
                "w2_d": w2_dev,
            }
        )
    return in_maps


def assemble_output(results):
    out = np.empty((B, D), np.float32)
    for c in range(NCORES):
        od = results[c]["out_d"]  # [128, KC*BL]
        outT = od.reshape(P, KC, BL).transpose(1, 0, 2).reshape(D, BL)
        out[c * BL : (c + 1) * BL, :] = outT.T
    return out


def kernel(**inputs):
    from concourse.bass_utils import run_bass_kernel_spmd

    nc = _get_nc()
    in_maps = prepare_in_maps(inputs)
    res = run_bass_kernel_spmd(nc, in_maps, core_ids=list(range(NCORES)))
    return assemble_output(res.results)



# revision 4
# speedup vs baseline: 1.1898x; 1.1898x over previous
"""ContinuousDeepFM Trainium2 kernel (8-core data-parallel over batch).

Math (algebraically collapsed from the reference — the [B,D,D] interaction
tensor is never materialized):
    fo  = x @ W1 + bias
    xw  = x @ W2
    so[b,j] = 0.5 * xw[b,j]^2 * t[b],  t[b] = sum_i x[b,i]^2 - (sum_i x[b,i])^2
    h   = MLP(x @ Wf)   (3 ReLU layers + final linear, weights mlp_w[i].T)
    out = fo + so + h

Sharding: batch 512 -> 64 rows per core; weights replicated. On-chip layout
is feature-major (activations stored transposed as 4 chunks of 128
partitions) so no on-chip transposes are needed; per-feature biases become
per-partition scalars. t depends only on x, so it is computed host-side in
fp64 and shipped pre-broadcast.

Precision: the output is dominated by the second-order term (RMS ~2e5 vs
~23 for fo and ~1 for h). W2 and the x used with it run in bf16 (measured
end-to-end rel err ~3e-3 vs the 2e-2 gate); the fo/deep weights and
activations run in fp8e4m3. x is pre-cast host-side to both dtypes so no
on-chip casts gate the first matmul.

Schedule: two HWDGE rings stream weights in compute order (wf, mw0..mw2 on
sync; x8, xb, th+bias, w2, mw3, w1 on scalar) so the Tensor engine runs
directly behind the arrivals: h0 -> l0 -> xw -> l1 -> l2 -> fo -> mw3@h3
(fo and the mw3 term share one PSUM accumulation group). The second-order
chain (square on Scalar, *t and +bias on Vector) runs during the deep
chain, ReLU drains alternate Vector/Scalar, final adds alternate
Vector/GpSimd, and the output is DMA'd per 128-feature chunk on both rings
as soon as each chunk's add completes.
"""

import numpy as np
import ml_dtypes

B = 512
D = 512
NCORES = 8
BL = B // NCORES  # 64 batch rows per core
P = 128
KC = D // P  # 4 partition chunks of the feature dim

F8 = ml_dtypes.float8_e4m3
BF16 = ml_dtypes.bfloat16

_NC_CACHE = {}


def _split_multi_waits(nc, mybir):
    """This container's walrus build supports only ONE sync wait per
    instruction, but Tile's scheduler attaches several (e.g. the exit
    drain). Split extras into preceding single-wait NoOps on the same
    engine — in-order execution preserves the barrier semantics."""
    ctr = 0
    for fn in nc.m.functions:
        for blk in fn.blocks:
            insts = blk.instructions
            if not any(
                i.sync_info is not None
                and i.sync_info.on_wait
                and len(i.sync_info.on_wait) > 1
                for i in insts
            ):
                continue
            out = []
            for inst in insts:
                si = inst.sync_info
                if si is not None and si.on_wait and len(si.on_wait) > 1:
                    waits = list(si.on_wait)
                    for w in waits[:-1]:
                        ctr += 1
                        nop = mybir.InstNoOp(
                            name=f"wsplit-{ctr}-{inst.name}", ins=[], outs=[]
                        )
                        nop.engine = inst.engine
                        nop.sync_info = mybir.SyncInfo(on_wait=[w], on_update=[])
                        out.append(nop)
                    si.on_wait = [waits[-1]]
                out.append(inst)
            blk.instructions = out
    return ctr


def _build_nc():
    import concourse.bass as bass
    import concourse.mybir as mybir
    import concourse.tile as tile

    dt = mybir.dt
    f32 = dt.float32
    f8 = dt.float8e4
    bf = dt.bfloat16
    Alu = mybir.AluOpType
    Act = mybir.ActivationFunctionType

    nc = bass.Bass("TRN2", target_bir_lowering=False, debug=False)

    x8_d = nc.dram_tensor("x8_d", [P, KC * BL], f8, kind="ExternalInput")
    xb_d = nc.dram_tensor("xb_d", [P, KC * BL], bf, kind="ExternalInput")
    thb_d = nc.dram_tensor("thb_d", [P, BL + 16], f32, kind="ExternalInput")
    wf_d = nc.dram_tensor("wf_d", [P, KC * D], f8, kind="ExternalInput")
    mw_d = nc.dram_tensor("mw_d", [P, 4 * KC * D], f8, kind="ExternalInput")
    w1_d = nc.dram_tensor("w1_d", [P, KC * D], f8, kind="ExternalInput")
    w2_d = nc.dram_tensor("w2_d", [P, KC * D], bf, kind="ExternalInput")
    out_d = nc.dram_tensor("out_d", [P, KC * BL], f32, kind="ExternalOutput")

    with tile.TileContext(nc) as tc:
        with (
            tc.tile_pool(name="w", bufs=1) as wpool,
            tc.tile_pool(name="act", bufs=1) as apool,
            tc.tile_pool(name="ps", bufs=1, space="PSUM") as pspool,
        ):
            # ---- input DMAs, one dense 2D copy each. Ring A (sync) carries
            # the deep-chain weights in consumption order; ring B (scalar)
            # carries the activations first, then w2/mw3/w1.
            wf_sb = wpool.tile([P, KC * D], f8, tag="wf")
            nc.sync.dma_start(wf_sb[:], wf_d.ap())
            mw_sb = wpool.tile([P, 4 * KC * D], f8, tag="mw")
            for i in range(3):
                nc.sync.dma_start(
                    mw_sb[:, i * KC * D : (i + 1) * KC * D],
                    mw_d.ap()[:, i * KC * D : (i + 1) * KC * D],
                )

            x8 = apool.tile([P, KC * BL], f8, tag="x8")
            nc.scalar.dma_start(x8[:], x8_d.ap())
            xb = apool.tile([P, KC * BL], bf, tag="xb")
            nc.scalar.dma_start(xb[:], xb_d.ap())
            thb = apool.tile([P, BL + 16], f32, tag="thb")
            nc.scalar.dma_start(thb[:], thb_d.ap())
            w2_sb = wpool.tile([P, KC * D], bf, tag="w2")
            nc.scalar.dma_start(w2_sb[:], w2_d.ap())
            nc.scalar.dma_start(
                mw_sb[:, 3 * KC * D : 4 * KC * D],
                mw_d.ap()[:, 3 * KC * D : 4 * KC * D],
            )
            w1_sb = wpool.tile([P, KC * D], f8, tag="w1")
            nc.scalar.dma_start(w1_sb[:], w1_d.ap())

            th = thb[:, 0:BL]
            bias_sb = thb[:, BL : BL + 16]

            def wsl(t, kc, jc, base=0):
                return t[:, base + kc * D + jc * P : base + kc * D + (jc + 1) * P]

            def xsl(t, kc):
                return t[:, kc * BL : (kc + 1) * BL]

            def mm_group(ps, w_t, rhs_t, base=0, start=True, stop=True):
                for kc in range(KC):
                    for jc in range(KC):
                        nc.tensor.matmul(
                            ps[jc][:],
                            wsl(w_t, kc, jc, base=base),
                            xsl(rhs_t, kc),
                            start=start and (kc == 0),
                            stop=stop and (kc == KC - 1),
                        )

            def psum_group(name):
                return [
                    pspool.tile([P, BL], f32, tag="mm", bufs=8, name=f"{name}p{j}")
                    for j in range(KC)
                ]

            # ---- deep chain (fp8): h0 = x @ Wf; drain on Vector (plain cast)
            h_ps = psum_group("h0")
            mm_group(h_ps, wf_sb, x8)
            h = apool.tile([P, KC * BL], f8, tag="h0")
            for jc in range(KC):
                nc.vector.tensor_copy(xsl(h, jc), h_ps[jc][:])

            # hidden layer 0
            l_ps = psum_group("l0")
            mm_group(l_ps, mw_sb, h, base=0)

            # ---- xw = x @ W2 (bf16), interleaved while mw1 streams in
            xw_ps = psum_group("xw")
            mm_group(xw_ps, w2_sb, xb)

            # l0 drain: ReLU+bias, alternating Vector / Scalar(Act)
            def relu_drain(ps, i):
                hn = apool.tile([P, KC * BL], f8, tag=f"h{i + 1}")
                for jc in range(KC):
                    bcol = 4 + i * KC + jc
                    if jc % 2 == 0:
                        nc.vector.tensor_scalar(
                            xsl(hn, jc),
                            ps[jc][:],
                            bias_sb[:, bcol : bcol + 1],
                            0.0,
                            op0=Alu.add,
                            op1=Alu.max,
                        )
                    else:
                        nc.scalar.activation(
                            xsl(hn, jc),
                            ps[jc][:],
                            Act.Relu,
                            bias=bias_sb[:, bcol : bcol + 1],
                        )
                return hn

            h = relu_drain(l_ps, 0)

            # second-order: xwsq = xw^2 on Scalar (PSUM->SBUF), then
            # so2 = xwsq*th + btot on GpSimd (SBUF-only; GpSimd can't
            # access PSUM on TRN2)
            xwsq = apool.tile([P, KC * BL], f32, tag="xwsq")
            for jc in range(KC):
                nc.scalar.square(xsl(xwsq, jc), xw_ps[jc][:])
            so = apool.tile([P, KC * BL], f32, tag="so")
            for jc in range(KC):
                nc.gpsimd.tensor_mul(xsl(so, jc), xsl(xwsq, jc), th)
            so2 = apool.tile([P, KC * BL], f32, tag="so2")
            for jc in range(KC):
                nc.gpsimd.tensor_scalar(
                    xsl(so2, jc),
                    xsl(so, jc),
                    bias_sb[:, jc : jc + 1],
                    None,
                    op0=Alu.add,
                )

            # hidden layers 1, 2
            l_ps = psum_group("l1")
            mm_group(l_ps, mw_sb, h, base=KC * D)
            h = relu_drain(l_ps, 1)

            l_ps = psum_group("l2")
            mm_group(l_ps, mw_sb, h, base=2 * KC * D)
            h = relu_drain(l_ps, 2)

            # ---- final PSUM group: o = x @ W1 + h3 @ mw[3].T
            o_ps = psum_group("o")
            mm_group(o_ps, w1_sb, x8, start=True, stop=False)
            mm_group(o_ps, mw_sb, h, base=3 * KC * D, start=False, stop=True)

            # out = o + so2 on Vector (PSUM read), each chunk DMA'd out as
            # soon as it's ready (ring B for jc 0/1, ring A for 2/3).
            out_sb = apool.tile([P, KC * BL], f32, tag="out")
            for jc in range(KC):
                nc.vector.tensor_add(xsl(out_sb, jc), o_ps[jc][:], xsl(so2, jc))
                ring = nc.scalar if jc < 2 else nc.sync
                ring.dma_start(
                    out_d.ap()[:, jc * BL : (jc + 1) * BL], xsl(out_sb, jc)
                )

    _split_multi_waits(nc, mybir)
    return nc


def _get_nc():
    if "nc" not in _NC_CACHE:
        _NC_CACHE["nc"] = _build_nc()
    return _NC_CACHE["nc"]


def _chunk_major(w):
    """[D, D] lhsT-layout weight -> dense [128, KC*D] chunk-major array."""
    return np.ascontiguousarray(
        w.reshape(KC, P, D).transpose(1, 0, 2).reshape(P, KC * D)
    )


def prepare_in_maps(inputs):
    x = np.asarray(inputs["x"], np.float32)
    w1 = np.asarray(inputs["first_order_weights"], np.float32)
    bias = np.asarray(inputs["bias"], np.float32)
    w2 = np.asarray(inputs["second_order_weights"], np.float32)
    wf = np.asarray(inputs["feature_weights"], np.float32)
    mw = np.asarray(inputs["mlp_w"], np.float32)
    mb = np.asarray(inputs["mlp_b"], np.float32)

    # t[b] = sum x^2 - (sum x)^2 (host, fp64), shipped as 0.5*t broadcast
    xd = x.astype(np.float64)
    t = (xd * xd).sum(1) - xd.sum(1) ** 2
    th_full = (0.5 * t).astype(np.float32)

    w2_dev = _chunk_major(w2).astype(BF16)
    wf_dev = _chunk_major(wf).astype(F8)
    w1_dev = _chunk_major(w1).astype(F8)
    # mw[i].T is the lhsT; layer-major, then chunk-major within each layer
    mwT = mw.transpose(0, 2, 1)  # [4, D(k), D(m)]
    mw_dev = np.ascontiguousarray(
        mwT.reshape(4, KC, P, D).transpose(2, 0, 1, 3).reshape(P, 4 * KC * D)
    ).astype(F8)
    # bias block layout: [btot(4) | mb0(4) | mb1(4) | mb2(4)]
    btot = (bias + mb[3]).astype(np.float32).reshape(KC, P).T  # [128, 4]
    mb3 = mb[:3].astype(np.float32).reshape(3, KC, P).transpose(2, 0, 1).reshape(P, 12)
    bias_dev = np.concatenate([btot, mb3], axis=1)  # [128, 16]

    in_maps = []
    for c in range(NCORES):
        xs = x[c * BL : (c + 1) * BL, :].T  # [512, 64]
        x_dev = np.ascontiguousarray(
            xs.reshape(KC, P, BL).transpose(1, 0, 2).reshape(P, KC * BL)
        )
        thb_dev = np.ascontiguousarray(
            np.concatenate(
                [
                    np.broadcast_to(th_full[c * BL : (c + 1) * BL], (P, BL)),
                    bias_dev,
                ],
                axis=1,
            )
        )
        in_maps.append(
            {
                "x8_d": x_dev.astype(F8),
                "xb_d": x_dev.astype(BF16),
                "thb_d": thb_dev,
                "wf_d": wf_dev,
                "mw_d": mw_dev,
                "w1_d": w1_dev,
                "w2_d": w2_dev,
            }
        )
    return in_maps


def assemble_output(results):
    out = np.empty((B, D), np.float32)
    for c in range(NCORES):
        od = results[c]["out_d"]  # [128, KC*BL]
        outT = od.reshape(P, KC, BL).transpose(1, 0, 2).reshape(D, BL)
        out[c * BL : (c + 1) * BL, :] = outT.T
    return out


def kernel(**inputs):
    from concourse.bass_utils import run_bass_kernel_spmd

    nc = _get_nc()
    in_maps = prepare_in_maps(inputs)
    res = run_bass_kernel_spmd(nc, in_maps, core_ids=list(range(NCORES)))
    return assemble_output(res.results)


# revision 6
# speedup vs baseline: 1.2132x; 1.0196x over previous
"""ContinuousDeepFM Trainium2 kernel (8-core data-parallel over batch).

Math (algebraically collapsed from the reference — the [B,D,D] interaction
tensor is never materialized):
    fo  = x @ W1 + bias
    xw  = x @ W2
    so[b,j] = 0.5 * xw[b,j]^2 * t[b],  t[b] = sum_i x[b,i]^2 - (sum_i x[b,i])^2
    h   = MLP(x @ Wf)   (3 ReLU layers + final linear, weights mlp_w[i].T)
    out = fo + so + h

Sharding: batch 512 -> 64 rows per core; weights replicated. On-chip layout
is feature-major (activations stored transposed as 4 chunks of 128
partitions) so no on-chip transposes are needed; per-feature biases become
per-partition scalars. t depends only on x, so it is computed host-side in
fp64 and shipped pre-broadcast.

Precision: the output is dominated by the second-order term (RMS ~2e5 vs
~23 for fo and ~1 for h). W2 and the x used with it run in bf16 (measured
end-to-end rel err ~3e-3 vs the 2e-2 gate); the fo/deep weights and
activations run in fp8e4m3. x is pre-cast host-side to both dtypes so no
on-chip casts gate the first matmul.

Schedule: two HWDGE rings stream weights in compute order (wf, mw0..mw2 on
sync; x8, xb, th+bias, w2, mw3, w1 on scalar) so the Tensor engine runs
directly behind the arrivals: h0 -> l0 -> xw -> l1 -> l2 -> fo -> mw3@h3
(fo and the mw3 term share one PSUM accumulation group). The second-order
chain (square on Scalar, *t and +bias on Vector) runs during the deep
chain, ReLU drains alternate Vector/Scalar, final adds alternate
Vector/GpSimd, and the output is DMA'd per 128-feature chunk on both rings
as soon as each chunk's add completes.
"""

import numpy as np
import ml_dtypes

B = 512
D = 512
NCORES = 8
BL = B // NCORES  # 64 batch rows per core
P = 128
KC = D // P  # 4 partition chunks of the feature dim

F8 = ml_dtypes.float8_e4m3
BF16 = ml_dtypes.bfloat16

_NC_CACHE = {}


def _split_multi_waits(nc, mybir):
    """This container's walrus build supports only ONE sync wait per
    instruction, but Tile's scheduler attaches several (e.g. the exit
    drain). Split extras into preceding single-wait NoOps on the same
    engine — in-order execution preserves the barrier semantics."""
    ctr = 0
    for fn in nc.m.functions:
        for blk in fn.blocks:
            insts = blk.instructions
            if not any(
                i.sync_info is not None
                and i.sync_info.on_wait
                and len(i.sync_info.on_wait) > 1
                for i in insts
            ):
                continue
            out = []
            for inst in insts:
                si = inst.sync_info
                if si is not None and si.on_wait and len(si.on_wait) > 1:
                    waits = list(si.on_wait)
                    for w in waits[:-1]:
                        ctr += 1
                        nop = mybir.InstNoOp(
                            name=f"wsplit-{ctr}-{inst.name}", ins=[], outs=[]
                        )
                        nop.engine = inst.engine
                        nop.sync_info = mybir.SyncInfo(on_wait=[w], on_update=[])
                        out.append(nop)
                    si.on_wait = [waits[-1]]
                out.append(inst)
            blk.instructions = out
    return ctr


def _build_nc():
    import concourse.bass as bass
    import concourse.mybir as mybir
    import concourse.tile as tile

    dt = mybir.dt
    f32 = dt.float32
    f8 = dt.float8e4
    bf = dt.bfloat16
    Alu = mybir.AluOpType
    Act = mybir.ActivationFunctionType

    nc = bass.Bass("TRN2", target_bir_lowering=False, debug=False)

    x8_d = nc.dram_tensor("x8_d", [P, KC * BL], f8, kind="ExternalInput")
    xb_d = nc.dram_tensor("xb_d", [P, KC * BL], bf, kind="ExternalInput")
    thb_d = nc.dram_tensor("thb_d", [P, BL + 16], f32, kind="ExternalInput")
    wf_d = nc.dram_tensor("wf_d", [P, KC * D], f8, kind="ExternalInput")
    mw_d = nc.dram_tensor("mw_d", [P, 4 * KC * D], f8, kind="ExternalInput")
    w1_d = nc.dram_tensor("w1_d", [P, KC * D], f8, kind="ExternalInput")
    w2_d = nc.dram_tensor("w2_d", [P, KC * D], bf, kind="ExternalInput")
    out_d = nc.dram_tensor("out_d", [P, KC * BL], f32, kind="ExternalOutput")

    with tile.TileContext(nc) as tc:
        with (
            tc.tile_pool(name="w", bufs=1) as wpool,
            tc.tile_pool(name="act", bufs=1) as apool,
            tc.tile_pool(name="ps", bufs=1, space="PSUM") as pspool,
        ):
            # ---- input DMAs, one dense 2D copy each. Ring A (sync) carries
            # the deep-chain weights in consumption order; ring B (scalar)
            # carries the activations first, then w2/mw3/w1.
            wf_sb = wpool.tile([P, KC * D], f8, tag="wf")
            nc.sync.dma_start(wf_sb[:], wf_d.ap())
            mw_sb = wpool.tile([P, 4 * KC * D], f8, tag="mw")
            for i in range(4):
                nc.sync.dma_start(
                    mw_sb[:, i * KC * D : (i + 1) * KC * D],
                    mw_d.ap()[:, i * KC * D : (i + 1) * KC * D],
                )

            x8 = apool.tile([P, KC * BL], f8, tag="x8")
            nc.scalar.dma_start(x8[:], x8_d.ap())
            xb = apool.tile([P, KC * BL], bf, tag="xb")
            nc.scalar.dma_start(xb[:], xb_d.ap())
            thb = apool.tile([P, BL + 16], f32, tag="thb")
            nc.scalar.dma_start(thb[:], thb_d.ap())
            w2_sb = wpool.tile([P, KC * D], bf, tag="w2")
            nc.scalar.dma_start(w2_sb[:], w2_d.ap())
            w1_sb = wpool.tile([P, KC * D], f8, tag="w1")
            nc.scalar.dma_start(w1_sb[:], w1_d.ap())

            th = thb[:, 0:BL]
            bias_sb = thb[:, BL : BL + 16]

            def wsl(t, kc, jc, base=0):
                return t[:, base + kc * D + jc * P : base + kc * D + (jc + 1) * P]

            def xsl(t, kc):
                return t[:, kc * BL : (kc + 1) * BL]

            def mm_group(ps, w_t, rhs_t, base=0, start=True, stop=True):
                for kc in range(KC):
                    for jc in range(KC):
                        nc.tensor.matmul(
                            ps[jc][:],
                            wsl(w_t, kc, jc, base=base),
                            xsl(rhs_t, kc),
                            start=start and (kc == 0),
                            stop=stop and (kc == KC - 1),
                        )

            def psum_group(name):
                return [
                    pspool.tile([P, BL], f32, tag="mm", bufs=8, name=f"{name}p{j}")
                    for j in range(KC)
                ]

            # ---- deep chain (fp8): h0 = x @ Wf; drain on Vector (plain cast)
            h_ps = psum_group("h0")
            mm_group(h_ps, wf_sb, x8)
            h = apool.tile([P, KC * BL], f8, tag="h0")
            for jc in range(KC):
                nc.vector.tensor_copy(xsl(h, jc), h_ps[jc][:])

            # hidden layer 0
            l_ps = psum_group("l0")
            mm_group(l_ps, mw_sb, h, base=0)

            # ---- xw = x @ W2 (bf16), interleaved while mw1 streams in
            xw_ps = psum_group("xw")
            mm_group(xw_ps, w2_sb, xb)

            # l0 drain: ReLU+bias, alternating Vector / Scalar(Act)
            def relu_drain(ps, i):
                hn = apool.tile([P, KC * BL], f8, tag=f"h{i + 1}")
                for jc in range(KC):
                    bcol = 4 + i * KC + jc
                    if jc % 2 == 0:
                        nc.vector.tensor_scalar(
                            xsl(hn, jc),
                            ps[jc][:],
                            bias_sb[:, bcol : bcol + 1],
                            0.0,
                            op0=Alu.add,
                            op1=Alu.max,
                        )
                    else:
                        nc.scalar.activation(
                            xsl(hn, jc),
                            ps[jc][:],
                            Act.Relu,
                            bias=bias_sb[:, bcol : bcol + 1],
                        )
                return hn

            h = relu_drain(l_ps, 0)

            # second-order: xwsq = xw^2 on Scalar (PSUM->SBUF), then
            # so2 = xwsq*th + btot on GpSimd (SBUF-only; GpSimd can't
            # access PSUM on TRN2)
            xwsq = apool.tile([P, KC * BL], f32, tag="xwsq")
            for jc in range(KC):
                nc.scalar.square(xsl(xwsq, jc), xw_ps[jc][:])
            so = apool.tile([P, KC * BL], f32, tag="so")
            for jc in range(KC):
                nc.gpsimd.tensor_mul(xsl(so, jc), xsl(xwsq, jc), th)
            so2 = apool.tile([P, KC * BL], f32, tag="so2")
            for jc in range(KC):
                nc.gpsimd.tensor_scalar(
                    xsl(so2, jc),
                    xsl(so, jc),
                    bias_sb[:, jc : jc + 1],
                    None,
                    op0=Alu.add,
                )

            # hidden layers 1, 2
            l_ps = psum_group("l1")
            mm_group(l_ps, mw_sb, h, base=KC * D)
            h = relu_drain(l_ps, 1)

            l_ps = psum_group("l2")
            mm_group(l_ps, mw_sb, h, base=2 * KC * D)
            h = relu_drain(l_ps, 2)

            # ---- final PSUM group: o = x @ W1 + h3 @ mw[3].T
            o_ps = psum_group("o")
            mm_group(o_ps, w1_sb, x8, start=True, stop=False)
            mm_group(o_ps, mw_sb, h, base=3 * KC * D, start=False, stop=True)

            # out = o + so2 on Vector (PSUM read); halves DMA'd out as soon
            # as each pair of adds lands (ring B for jc 0/1, ring A for 2/3).
            out_sb = apool.tile([P, KC * BL], f32, tag="out")
            for jc in range(KC):
                nc.vector.tensor_add(xsl(out_sb, jc), o_ps[jc][:], xsl(so2, jc))
                if jc == 1:
                    nc.scalar.dma_start(
                        out_d.ap()[:, 0 : 2 * BL], out_sb[:, 0 : 2 * BL]
                    )
                elif jc == 3:
                    nc.sync.dma_start(
                        out_d.ap()[:, 2 * BL : 4 * BL], out_sb[:, 2 * BL : 4 * BL]
                    )

    _split_multi_waits(nc, mybir)
    return nc


def _get_nc():
    if "nc" not in _NC_CACHE:
        _NC_CACHE["nc"] = _build_nc()
    return _NC_CACHE["nc"]


def _chunk_major(w):
    """[D, D] lhsT-layout weight -> dense [128, KC*D] chunk-major array."""
    return np.ascontiguousarray(
        w.reshape(KC, P, D).transpose(1, 0, 2).reshape(P, KC * D)
    )


def prepare_in_maps(inputs):
    x = np.asarray(inputs["x"], np.float32)
    w1 = np.asarray(inputs["first_order_weights"], np.float32)
    bias = np.asarray(inputs["bias"], np.float32)
    w2 = np.asarray(inputs["second_order_weights"], np.float32)
    wf = np.asarray(inputs["feature_weights"], np.float32)
    mw = np.asarray(inputs["mlp_w"], np.float32)
    mb = np.asarray(inputs["mlp_b"], np.float32)

    # t[b] = sum x^2 - (sum x)^2 (host, fp64), shipped as 0.5*t broadcast
    xd = x.astype(np.float64)
    t = (xd * xd).sum(1) - xd.sum(1) ** 2
    th_full = (0.5 * t).astype(np.float32)

    w2_dev = _chunk_major(w2).astype(BF16)
    wf_dev = _chunk_major(wf).astype(F8)
    w1_dev = _chunk_major(w1).astype(F8)
    # mw[i].T is the lhsT; layer-major, then chunk-major within each layer
    mwT = mw.transpose(0, 2, 1)  # [4, D(k), D(m)]
    mw_dev = np.ascontiguousarray(
        mwT.reshape(4, KC, P, D).transpose(2, 0, 1, 3).reshape(P, 4 * KC * D)
    ).astype(F8)
    # bias block layout: [btot(4) | mb0(4) | mb1(4) | mb2(4)]
    btot = (bias + mb[3]).astype(np.float32).reshape(KC, P).T  # [128, 4]
    mb3 = mb[:3].astype(np.float32).reshape(3, KC, P).transpose(2, 0, 1).reshape(P, 12)
    bias_dev = np.concatenate([btot, mb3], axis=1)  # [128, 16]

    in_maps = []
    for c in range(NCORES):
        xs = x[c * BL : (c + 1) * BL, :].T  # [512, 64]
        x_dev = np.ascontiguousarray(
            xs.reshape(KC, P, BL).transpose(1, 0, 2).reshape(P, KC * BL)
        )
        thb_dev = np.ascontiguousarray(
            np.concatenate(
                [
                    np.broadcast_to(th_full[c * BL : (c + 1) * BL], (P, BL)),
                    bias_dev,
                ],
                axis=1,
            )
        )
        in_maps.append(
            {
                "x8_d": x_dev.astype(F8),
                "xb_d": x_dev.astype(BF16),
                "thb_d": thb_dev,
                "wf_d": wf_dev,
                "mw_d": mw_dev,
                "w1_d": w1_dev,
                "w2_d": w2_dev,
            }
        )
    return in_maps


def assemble_output(results):
    out = np.empty((B, D), np.float32)
    for c in range(NCORES):
        od = results[c]["out_d"]  # [128, KC*BL]
        outT = od.reshape(P, KC, BL).transpose(1, 0, 2).reshape(D, BL)
        out[c * BL : (c + 1) * BL, :] = outT.T
    return out


def kernel(**inputs):
    from concourse.bass_utils import run_bass_kernel_spmd

    nc = _get_nc()
    in_maps = prepare_in_maps(inputs)
    res = run_bass_kernel_spmd(nc, in_maps, core_ids=list(range(NCORES)))
    return assemble_output(res.results)


# revision 8
# speedup vs baseline: 1.2510x; 1.0312x over previous
"""ContinuousDeepFM Trainium2 kernel (8-core data-parallel over batch).

Math (algebraically collapsed from the reference — the [B,D,D] interaction
tensor is never materialized):
    fo  = x @ W1 + bias
    xw  = x @ W2
    so[b,j] = 0.5 * xw[b,j]^2 * t[b],  t[b] = sum_i x[b,i]^2 - (sum_i x[b,i])^2
    h   = MLP(x @ Wf)   (3 ReLU layers + final linear, weights mlp_w[i].T)
    out = fo + so + h

Sharding: batch 512 -> 64 rows per core; weights replicated. On-chip layout
is feature-major (activations stored transposed as 4 chunks of 128
partitions) so no on-chip transposes are needed; per-feature biases become
per-partition scalars. t depends only on x, so it is computed host-side in
fp64 and shipped pre-broadcast.

Precision: the output is dominated by the second-order term (RMS ~2e5 vs
~23 for fo and ~1 for h). W2 and the x used with it run in bf16 (measured
end-to-end rel err ~3e-3 vs the 2e-2 gate); the fo/deep weights and
activations run in fp8e4m3. x is pre-cast host-side to both dtypes so no
on-chip casts gate the first matmul.

Schedule notes (from NTFF traces): the two HWDGE rings share one pool of 16
DMA engines (~205-280 GB/s aggregate), so ring assignment is about arrival
ORDER, not bandwidth. Each PSUM accumulation group lives in a single bank
as one [128, 256] tile so post-matmul elementwise work is one instruction
instead of four (per-instruction overhead ~200ns dominates 64-element
ops). The final bias (bias + mlp_b[3]) enters the last PSUM group via
rank-1 fp32 matmuls (contraction dim 1 against a ones vector), which takes
it off the Vector/GpSimd critical tail entirely; GpSimd tensor_scalar is
~1.1us/op on this part and must never sit on the tail. The output is one
128KB DMA issued as soon as the single fused add lands.
"""

import numpy as np
import ml_dtypes

B = 512
D = 512
NCORES = 8
BL = B // NCORES  # 64 batch rows per core
P = 128
KC = D // P  # 4 partition chunks of the feature dim

F8 = ml_dtypes.float8_e4m3
BF16 = ml_dtypes.bfloat16

_NC_CACHE = {}


def _split_multi_waits(nc, mybir):
    """This container's walrus build supports only ONE sync wait per
    instruction, but Tile's scheduler attaches several (e.g. the exit
    drain). Split extras into preceding single-wait NoOps on the same
    engine — in-order execution preserves the barrier semantics."""
    ctr = 0
    for fn in nc.m.functions:
        for blk in fn.blocks:
            insts = blk.instructions
            if not any(
                i.sync_info is not None
                and i.sync_info.on_wait
                and len(i.sync_info.on_wait) > 1
                for i in insts
            ):
                continue
            out = []
            for inst in insts:
                si = inst.sync_info
                if si is not None and si.on_wait and len(si.on_wait) > 1:
                    waits = list(si.on_wait)
                    for w in waits[:-1]:
                        ctr += 1
                        nop = mybir.InstNoOp(
                            name=f"wsplit-{ctr}-{inst.name}", ins=[], outs=[]
                        )
                        nop.engine = inst.engine
                        nop.sync_info = mybir.SyncInfo(on_wait=[w], on_update=[])
                        out.append(nop)
                    si.on_wait = [waits[-1]]
                out.append(inst)
            blk.instructions = out
    return ctr


def _build_nc():
    import concourse.bass as bass
    import concourse.mybir as mybir
    import concourse.tile as tile

    dt = mybir.dt
    f32 = dt.float32
    f8 = dt.float8e4
    bf = dt.bfloat16
    Alu = mybir.AluOpType
    Act = mybir.ActivationFunctionType

    nc = bass.Bass("TRN2", target_bir_lowering=False, debug=False)

    x8_d = nc.dram_tensor("x8_d", [P, KC * BL], f8, kind="ExternalInput")
    xb_d = nc.dram_tensor("xb_d", [P, KC * BL], bf, kind="ExternalInput")
    thb_d = nc.dram_tensor("thb_d", [P, BL + 16], f32, kind="ExternalInput")
    bt_d = nc.dram_tensor("bt_d", [1, D], f32, kind="ExternalInput")
    wf_d = nc.dram_tensor("wf_d", [P, KC * D], f8, kind="ExternalInput")
    mw_d = nc.dram_tensor("mw_d", [P, 4 * KC * D], f8, kind="ExternalInput")
    w1_d = nc.dram_tensor("w1_d", [P, KC * D], f8, kind="ExternalInput")
    w2_d = nc.dram_tensor("w2_d", [P, KC * D], bf, kind="ExternalInput")
    out_d = nc.dram_tensor("out_d", [P, KC * BL], f32, kind="ExternalOutput")

    with tile.TileContext(nc) as tc:
        with (
            tc.tile_pool(name="w", bufs=1) as wpool,
            tc.tile_pool(name="act", bufs=1) as apool,
            tc.tile_pool(name="ps", bufs=1, space="PSUM") as pspool,
        ):
            # ---- input DMAs. Ring A (sync) carries the deep-chain weights
            # in consumption order; ring B (scalar) carries activations +
            # w2/w1. The rings share DMA engines, so this is ordering only.
            wf_sb = wpool.tile([P, KC * D], f8, tag="wf")
            nc.sync.dma_start(wf_sb[:], wf_d.ap())
            mw_sb = wpool.tile([P, 4 * KC * D], f8, tag="mw")
            for i in range(4):
                nc.sync.dma_start(
                    mw_sb[:, i * KC * D : (i + 1) * KC * D],
                    mw_d.ap()[:, i * KC * D : (i + 1) * KC * D],
                )

            x8 = apool.tile([P, KC * BL], f8, tag="x8")
            nc.scalar.dma_start(x8[:], x8_d.ap())
            xb = apool.tile([P, KC * BL], bf, tag="xb")
            nc.scalar.dma_start(xb[:], xb_d.ap())
            thb = apool.tile([P, BL + 16], f32, tag="thb")
            nc.scalar.dma_start(thb[:], thb_d.ap())
            bt_sb = apool.tile([1, D], f32, tag="bt")
            nc.scalar.dma_start(bt_sb[:], bt_d.ap())
            w2_sb = wpool.tile([P, KC * D], bf, tag="w2")
            nc.scalar.dma_start(w2_sb[:], w2_d.ap())
            w1_sb = wpool.tile([P, KC * D], f8, tag="w1")
            nc.scalar.dma_start(w1_sb[:], w1_d.ap())

            th = thb[:, 0:BL]
            bias_sb = thb[:, BL : BL + 16]

            # ones vector for the rank-1 bias matmuls
            ones = apool.tile([1, BL], f32, tag="ones")
            nc.gpsimd.memset(ones[:], 1.0)

            def wsl(t, kc, jc, base=0):
                return t[:, base + kc * D + jc * P : base + kc * D + (jc + 1) * P]

            def xsl(t, kc):
                return t[:, kc * BL : (kc + 1) * BL]

            def mm_group(ps, w_t, rhs_t, base=0, start=True, stop=True):
                # The group shares ONE PSUM bank: start zeroes the whole
                # bank, so only the group's first matmul may carry it (the
                # rest accumulate onto the zeroed bank); stop only on last.
                for kc in range(KC):
                    for jc in range(KC):
                        nc.tensor.matmul(
                            xsl(ps, jc),
                            wsl(w_t, kc, jc, base=base),
                            xsl(rhs_t, kc),
                            start=start and (kc == 0) and (jc == 0),
                            stop=stop and (kc == KC - 1) and (jc == KC - 1),
                        )

            def psum_group(name):
                # one bank per group: [128, 256] f32 = 1KB/partition
                return pspool.tile([P, KC * BL], f32, tag="mm", bufs=8, name=name)

            # ---- deep chain (fp8): h0 = x @ Wf; single fused drain
            h_ps = psum_group("h0")
            mm_group(h_ps, wf_sb, x8)
            h = apool.tile([P, KC * BL], f8, tag="h0")
            nc.vector.tensor_copy(h[:], h_ps[:])

            # l0 drain: ReLU+bias, per-chunk bias => 4 ops, split V/S
            def relu_drain(ps, i):
                hn = apool.tile([P, KC * BL], f8, tag=f"h{i + 1}")
                for jc in range(KC):
                    bcol = 4 + i * KC + jc
                    if jc % 2 == 0:
                        nc.vector.tensor_scalar(
                            xsl(hn, jc),
                            xsl(ps, jc),
                            bias_sb[:, bcol : bcol + 1],
                            0.0,
                            op0=Alu.add,
                            op1=Alu.max,
                        )
                    else:
                        nc.scalar.activation(
                            xsl(hn, jc),
                            xsl(ps, jc),
                            Act.Relu,
                            bias=bias_sb[:, bcol : bcol + 1],
                        )
                return hn

            l_ps = psum_group("l0")
            mm_group(l_ps, mw_sb, h, base=0)
            h = relu_drain(l_ps, 0)

            l_ps = psum_group("l1")
            mm_group(l_ps, mw_sb, h, base=KC * D)
            h = relu_drain(l_ps, 1)

            # ---- xw = x @ W2 (bf16); square in ONE Scalar op, *th on GpSimd
            xw_ps = psum_group("xw")
            mm_group(xw_ps, w2_sb, xb)
            xwsq = apool.tile([P, KC * BL], f32, tag="xwsq")
            nc.scalar.square(xwsq[:], xw_ps[:])
            so = apool.tile([P, KC * BL], f32, tag="so")
            for jc in range(KC):
                nc.gpsimd.tensor_mul(xsl(so, jc), xsl(xwsq, jc), th)

            l_ps = psum_group("l2")
            mm_group(l_ps, mw_sb, h, base=2 * KC * D)
            h = relu_drain(l_ps, 2)

            # ---- final PSUM group: o = x@W1 + btot (rank-1) + h3@mw[3].T
            o_ps = psum_group("o")
            mm_group(o_ps, w1_sb, x8, start=True, stop=False)
            for jc in range(KC):
                nc.tensor.matmul(
                    xsl(o_ps, jc),
                    bt_sb[0:1, jc * P : (jc + 1) * P],
                    ones[0:1, :],
                    start=False,
                    stop=False,
                )
            mm_group(o_ps, mw_sb, h, base=3 * KC * D, start=False, stop=True)

            # out = o + so in ONE Vector op, then one 128KB output DMA
            out_sb = apool.tile([P, KC * BL], f32, tag="out")
            nc.vector.tensor_add(out_sb[:], o_ps[:], so[:])
            nc.scalar.dma_start(out_d.ap(), out_sb[:])

    _split_multi_waits(nc, mybir)
    return nc


def _get_nc():
    if "nc" not in _NC_CACHE:
        _NC_CACHE["nc"] = _build_nc()
    return _NC_CACHE["nc"]


def _chunk_major(w):
    """[D, D] lhsT-layout weight -> dense [128, KC*D] chunk-major array."""
    return np.ascontiguousarray(
        w.reshape(KC, P, D).transpose(1, 0, 2).reshape(P, KC * D)
    )


def prepare_in_maps(inputs):
    x = np.asarray(inputs["x"], np.float32)
    w1 = np.asarray(inputs["first_order_weights"], np.float32)
    bias = np.asarray(inputs["bias"], np.float32)
    w2 = np.asarray(inputs["second_order_weights"], np.float32)
    wf = np.asarray(inputs["feature_weights"], np.float32)
    mw = np.asarray(inputs["mlp_w"], np.float32)
    mb = np.asarray(inputs["mlp_b"], np.float32)

    # t[b] = sum x^2 - (sum x)^2 (host, fp64), shipped as 0.5*t broadcast
    xd = x.astype(np.float64)
    t = (xd * xd).sum(1) - xd.sum(1) ** 2
    th_full = (0.5 * t).astype(np.float32)

    w2_dev = _chunk_major(w2).astype(BF16)
    wf_dev = _chunk_major(wf).astype(F8)
    w1_dev = _chunk_major(w1).astype(F8)
    # mw[i].T is the lhsT; layer-major, then chunk-major within each layer
    mwT = mw.transpose(0, 2, 1)  # [4, D(k), D(m)]
    mw_dev = np.ascontiguousarray(
        mwT.reshape(4, KC, P, D).transpose(2, 0, 1, 3).reshape(P, 4 * KC * D)
    ).astype(F8)
    # final bias (bias + mlp_b[3]) as a [1, D] row for the rank-1 matmuls,
    # in chunk-major feature order to match the output layout
    btot = (bias + mb[3]).astype(np.float32).reshape(1, D)
    # per-layer MLP biases as per-partition columns: [mb0(4) | mb1(4) | mb2(4)]
    mb3 = mb[:3].astype(np.float32).reshape(3, KC, P).transpose(2, 0, 1).reshape(P, 12)
    bias_dev = np.concatenate([np.zeros((P, 4), np.float32), mb3], axis=1)

    in_maps = []
    for c in range(NCORES):
        xs = x[c * BL : (c + 1) * BL, :].T  # [512, 64]
        x_dev = np.ascontiguousarray(
            xs.reshape(KC, P, BL).transpose(1, 0, 2).reshape(P, KC * BL)
        )
        thb_dev = np.ascontiguousarray(
            np.concatenate(
                [
                    np.broadcast_to(th_full[c * BL : (c + 1) * BL], (P, BL)),
                    bias_dev,
                ],
                axis=1,
            )
        )
        in_maps.append(
            {
                "x8_d": x_dev.astype(F8),
                "xb_d": x_dev.astype(BF16),
                "thb_d": thb_dev,
                "bt_d": btot,
                "wf_d": wf_dev,
                "mw_d": mw_dev,
                "w1_d": w1_dev,
                "w2_d": w2_dev,
            }
        )
    return in_maps


def assemble_output(results):
    out = np.empty((B, D), np.float32)
    for c in range(NCORES):
        od = results[c]["out_d"]  # [128, KC*BL]
        outT = od.reshape(P, KC, BL).transpose(1, 0, 2).reshape(D, BL)
        out[c * BL : (c + 1) * BL, :] = outT.T
    return out


def kernel(**inputs):
    from concourse.bass_utils import run_bass_kernel_spmd

    nc = _get_nc()
    in_maps = prepare_in_maps(inputs)
    res = run_bass_kernel_spmd(nc, in_maps, core_ids=list(range(NCORES)))
    return assemble_output(res.results)
